# revision 1
# baseline (speedup 1.0000x reference)
"""Trainium2 Bass kernel for nn_L2Error_15539191677466 (vq_codebook).

Computes, for ze (B=8, Q=128, N=8192) and codebook emb (K=512, Q=128):

    out[b, n] = min_k sum_q (ze[b, q, n] - emb[k, q])**2
              = ze_sq[b, n] + emb_sq[k] - 2 * dot[b, k, n]  minimized over k

Sharding: data-parallel over B across the 8 NeuronCores (1 batch row per
core); the small codebook is replicated on every core.

Per-core algorithm (fp32r matmuls, fp32 accumulate/reduce):
  - zeb loads via cast-DMA to f32r [Q=128part, N]; emb is PE-transposed
    to embTs = -2*emb.T [Q, K=512] (f32r).
  - emb_sq row via all-ones matmul over (emb.T)^2; ze_sq rows via
    ones-column matmuls over zeb^2.
  - Both biases are folded into the PSUM grid with a rank-2 matmul:
    lhsT = [ze_sq[n]; 1], rhs = [1; emb_sq[k]], accumulated with the main
    matmul (stationary = zeb n-tile, moving = embTs) into [128n, 512k].
  - One grouped DVE tensor_reduce(min) per 2-3 PSUM banks produces the
    final minima directly; PE-transpose + store.
"""

import os
import sys
from contextlib import ExitStack

import numpy as np

for _p in ("/opt/trn_rl_repo", "/root/.axon_site/_ro/trn_rl_repo"):
    if os.path.isdir(_p) and _p not in sys.path:
        sys.path.append(_p)

import concourse.mybir as mybir  # noqa: E402
import concourse.tile as tile  # noqa: E402
from concourse import bacc  # noqa: E402
from concourse.bass_utils import run_bass_kernel_spmd  # noqa: E402
from concourse.masks import make_identity  # noqa: E402

B, Q, N, K = 8, 128, 8192, 512
P = 128
NT = N // P  # 64 n-tiles per core
F32 = mybir.dt.float32
F32R = mybir.dt.float32r
GROUPS = [3] * 20 + [2] * 2  # 64 n-tiles; 3-bank reduce groups (ragged tail)
GMAX = max(GROUPS)


def _build_kernel(ctx: ExitStack, tc: tile.TileContext, ze_d, emb_d, out_d, nc_top):
    nc = tc.nc

    const = ctx.enter_context(tc.tile_pool(name="const", bufs=1))
    zpool = ctx.enter_context(tc.tile_pool(name="zeb", bufs=1))
    gpsum = ctx.enter_context(tc.tile_pool(name="gpsum", bufs=2, space="PSUM"))
    mpsum = ctx.enter_context(tc.tile_pool(name="mpsum", bufs=1, space="PSUM"))

    ones_dram = nc_top.inline_tensor(np.ones((1, N), np.float32), name="onesrow").ap()

    ident = const.tile([P, P], F32)
    make_identity(nc, ident)
    ones = const.tile([P, P], F32)
    nc.gpsimd.memset(ones[:], 1.0)
    ones_r = const.tile([P, P], F32R)
    nc.scalar.copy(ones_r[:], ones[:])

    # --- emb (K, Q) -> transposed chunks: embTs = -2*emb.T (f32r), embT2 = (emb.T)^2
    emb_sb = const.tile([P, 4, P], F32)
    nc.sync.dma_start(emb_sb[:], emb_d.rearrange("(c p) q -> p c q", p=P))
    embTs = const.tile([P, K], F32R)
    embT2 = const.tile([P, K], F32)
    for c in range(4):
        tp = mpsum.tile([P, K], F32, tag="mp")
        nc.tensor.transpose(tp[:, 0:P], emb_sb[:, c], ident[:])
        nc.scalar.mul(embTs[:, c * P : (c + 1) * P], tp[:, 0:P], -2.0)
        nc.scalar.square(embT2[:, c * P : (c + 1) * P], tp[:, 0:P])

    # --- emb_sq row: ones.T @ embT2 -> every partition holds the row; take row 0
    ebc = mpsum.tile([P, K], F32, tag="mp")
    nc.tensor.matmul(ebc[:], ones[:], embT2[:], start=True, stop=True)
    tmpr = const.tile([1, K], F32)
    nc.scalar.copy(tmpr[:], ebc[0:1, :])

    # --- bias moving operand: [1; emb_sq[k]] (f32r)
    brs = const.tile([2, K], F32)
    nc.sync.dma_start(brs[0:1, :], ones_dram[0:1, 0:K])
    nc.sync.dma_start(brs[1:2, :], tmpr[:])
    brhs = const.tile([2, K], F32R)
    nc.scalar.copy(brhs[:], brs[:])

    # --- zeb: cast-DMA straight to f32r; squares (f32r) for ze_sq
    zeb = zpool.tile([P, N], F32R)
    zeb2 = zpool.tile([P, N], F32R)
    CH = 2048
    for i in range(N // CH):
        sl = slice(i * CH, (i + 1) * CH)
        nc.gpsimd.dma_start(zeb[:, sl], ze_d[:, sl])
        nc.scalar.square(zeb2[:, sl], zeb[:, sl])

    # --- bias stationary operand: [ze_sq[n]; 1] (f32r), built in 512-wide chunks
    bls = const.tile([2, N], F32)
    nc.sync.dma_start(bls[1:2, :], ones_dram[0:1, 0:N])
    blhsT = const.tile([2, N], F32R)
    for s in range(N // K):
        sl = slice(s * K, (s + 1) * K)
        zrow = mpsum.tile([P, K], F32, tag="zrow")
        nc.tensor.matmul(zrow[:], ones_r[:], zeb2[:, sl], start=True, stop=True)
        nc.scalar.copy(bls[0:1, sl], zrow[0:1, :])
        nc.scalar.copy(blhsT[:, sl], bls[:, sl])

    # --- main: per n-tile, rank-2 bias matmul + main matmul into one PSUM
    # bank; grouped min-reduce over 2-3 banks at a time
    minacc = const.tile([P, NT], F32)
    j = 0
    for gs in GROUPS:
        g = gpsum.tile([P, GMAX, K], F32, tag="grid")
        for jj in range(gs):
            sl = slice((j + jj) * P, (j + jj + 1) * P)
            nc.tensor.matmul(
                g[:, jj, :], blhsT[:, sl], brhs[:], start=True, stop=False
            )
            nc.tensor.matmul(g[:, jj, :], zeb[:, sl], embTs[:], start=False, stop=True)
        nc.vector.tensor_reduce(
            minacc[:, j : j + gs],
            g[:, 0:gs, :],
            axis=mybir.AxisListType.X,
            op=mybir.AluOpType.min,
        )
        j += gs

    # --- transpose [128p, 64j] -> [64j, 128p] and store n-major
    tpo = mpsum.tile([P, K], F32, tag="mp")
    nc.tensor.transpose(tpo[0:NT, 0:P], minacc[:], ident[:])
    bounce = const.tile([NT, P], F32)
    nc.scalar.copy(bounce[:], tpo[0:NT, 0:P])
    nc.sync.dma_start(out_d.rearrange("(j p) -> j p", p=P), bounce[:])


_NC_CACHE = None


def _get_nc():
    global _NC_CACHE
    if _NC_CACHE is None:
        nc = bacc.Bacc("TRN2", target_bir_lowering=False, debug=False)
        ze_d = nc.dram_tensor("ze_b", [Q, N], F32, kind="ExternalInput").ap()
        emb_d = nc.dram_tensor("emb", [K, Q], F32, kind="ExternalInput").ap()
        out_d = nc.dram_tensor("out", [N], F32, kind="ExternalOutput").ap()
        with tile.TileContext(nc) as tc, ExitStack() as ctx:
            _build_kernel(ctx, tc, ze_d, emb_d, out_d, nc)
        nc.compile()
        _NC_CACHE = nc
    return _NC_CACHE


def kernel(ze: np.ndarray, emb: np.ndarray) -> np.ndarray:
    ze = np.ascontiguousarray(np.asarray(ze, dtype=np.float32))
    emb = np.ascontiguousarray(np.asarray(emb, dtype=np.float32))
    assert ze.shape == (B, Q, N) and emb.shape == (K, Q)
    nc = _get_nc()
    in_maps = [{"ze_b": ze[b], "emb": emb} for b in range(B)]
    res = run_bass_kernel_spmd(nc, in_maps, core_ids=list(range(B)))
    return np.stack([res.results[b]["out"] for b in range(B)], axis=0)

